# revision 45
# baseline (speedup 1.0000x reference)
"""Trainium2 Bass kernel for nn_GraphPool (batched attentive FPS graph pooling).

Contract: kernel(**inputs) takes FULL inputs (B=128 graphs), shards the batch
dim across 8 NeuronCores (16 graphs each, pure data parallel), runs one SPMD
Bass program, and returns the FULL [128, 512] output.

Per-core algorithm (G=16 graphs, N=256 nodes, H=512, NH=8 heads, K=5):
  scores[g,j] = sum_{h, i<m} attn[g,h,i,j]  -> PE matmuls with block-diagonal
      0/1 mask weights (lhsT [128, 16], one nonzero column per graph) so all
      16 graphs accumulate into ONE psum tile [16, 256]; attn is DMA'd with
      row-pair interleave (i = 2p+t) giving 2KB descriptors, 2 graphs per DMA,
      masked via even/odd parity mask columns.
  sp: load row-pair chunks, row-mask on gpsimd, PE-transpose into a single
      PSUM bank [128, 512], one wide ACT copy out, fused 3D colmax -> dmax;
      spT staged to DRAM for the FPS column gathers (as row gathers).
  FPS in the dmax-scaled domain (cand' = cand*dmax): candAcc = min-chain with
      the bonus folded in; -BIG marks for invalid/selected nodes live in
      bonusM and ride every min-update; per iteration: max/max_index ->
      indirect gather of the selected sp column -> min.
  pool: x rows gathered with the same offsets, accumulated in-flight via
      DMA compute_op=add; LayerNorm via bn_stats/bn_aggr (eps scaled by K^2).

All mask/iota constants are precomputed on the host and passed as extra
inputs — deriving them on-chip serialized the first ~35us of the kernel.
(tensor_tensor_reduce and indirect compute_op min/max are avoided: rejected
or exec-unit-wedging on this HW; compute_op=add is fine.)
"""

import os
import sys
from contextlib import ExitStack

for _p in ("/opt/trn_rl_repo", "/root/.axon_site/_ro/trn_rl_repo"):
    if os.path.isdir(_p) and _p not in sys.path:
        sys.path.append(_p)

import numpy as np

import concourse.mybir as mybir
from concourse.bass import Bass, IndirectOffsetOnAxis
from concourse.bacc import Bacc
from concourse.masks import make_identity
from concourse.tile import TileContext

B, N, H, NH, K = 128, 256, 512, 8, 5
NCORES = 8
G = B // NCORES  # graphs per core
P = 128
LN_EPS = 1e-5
BIG = 1.0e30  # unavailable-node mark

f32 = mybir.dt.float32
f32r = mybir.dt.float32r
i32 = mybir.dt.int32
u32 = mybir.dt.uint32
AX = mybir.AxisListType
OP = mybir.AluOpType

TRACE = False
LAST_RESULT = None
GPER = 2  # graphs per attn DMA


def build_bass() -> Bass:
    nc = Bacc()
    x = nc.dram_tensor("x", [G, N, H], f32, kind="ExternalInput")
    attn = nc.dram_tensor("attn", [G, NH, N, N], f32, kind="ExternalInput")
    sp = nc.dram_tensor("spatial_pos", [G, N, N], f32, kind="ExternalInput")
    xm = nc.dram_tensor("x_mask", [G, N], f32, kind="ExternalInput")
    # host-precomputed constants (see core_inputs)
    xbde_d = nc.dram_tensor("c_xbde", [P, G * G], f32r, kind="ExternalInput")
    xbdo_d = nc.dram_tensor("c_xbdo", [P, G * G], f32r, kind="ExternalInput")
    xmt2_d = nc.dram_tensor("c_xmt2", [P, 2 * G], f32, kind="ExternalInput")
    nmpre_d = nc.dram_tensor("c_nmpre", [G, N], f32, kind="ExternalInput")
    iotaf_d = nc.dram_tensor("c_iotaf", [G, N], f32, kind="ExternalInput")
    rowbi_d = nc.dram_tensor("c_rowbi", [G, 1], i32, kind="ExternalInput")
    rowbf_d = nc.dram_tensor("c_rowbf", [G, 1], f32, kind="ExternalInput")
    gb_d = nc.dram_tensor("c_gb", [G, H], f32, kind="ExternalInput")
    bb_d = nc.dram_tensor("c_bb", [G, H], f32, kind="ExternalInput")
    out = nc.dram_tensor("out", [G, H], f32, kind="ExternalOutput")
    spt_dram = nc.dram_tensor("spt_scratch", [G, N, N], f32, kind="Internal")

    x_flat = x[:].rearrange("g n h -> (g n) h")
    spt_flat = spt_dram[:].rearrange("g n j -> (g n) j")

    with TileContext(nc) as tc, ExitStack() as ctx:
        cpool = ctx.enter_context(tc.tile_pool(name="cpool", bufs=1))
        small = ctx.enter_context(tc.tile_pool(name="small", bufs=2))
        fps = ctx.enter_context(tc.tile_pool(name="fps", bufs=2))
        attn_pool = ctx.enter_context(tc.tile_pool(name="attn_pool", bufs=4))
        sp_pool = ctx.enter_context(tc.tile_pool(name="sp_pool", bufs=6))
        spt_pool = ctx.enter_context(tc.tile_pool(name="spt_pool", bufs=4))
        psum_sc = ctx.enter_context(tc.tile_pool(name="psum_sc", bufs=1, space="PSUM"))
        psum_tr = ctx.enter_context(tc.tile_pool(name="psum_tr", bufs=4, space="PSUM"))
        psum_mi = ctx.enter_context(tc.tile_pool(name="psum_mi", bufs=1, space="PSUM"))
        psum_fi = ctx.enter_context(tc.tile_pool(name="psum_fi", bufs=2, space="PSUM"))

        # ---- constant loads (critical ones first on sync, rest on scalar) ----
        XBDe = cpool.tile([P, G * G], f32r)
        nc.sync.dma_start(XBDe, xbde_d[:, :])
        XBDo = cpool.tile([P, G * G], f32r)
        nc.sync.dma_start(XBDo, xbdo_d[:, :])
        XBD = (XBDe, XBDo)

        xmT2 = cpool.tile([P, 2 * G], f32)
        nc.scalar.dma_start(xmT2, xmt2_d[:, :])

        ident = cpool.tile([P, P], f32)
        make_identity(nc, ident)

        CMall = cpool.tile([P, 2 * G], f32)
        scores_ps = psum_sc.tile([G, N], f32)

        def pe_filler(n):
            # Dummy transposes (PE-local, never read): keep the PE activity
            # monitor's clock gate at 8/8 through DMA waits so real matmuls
            # run at 2.4 GHz instead of 1.2 (HAM oscillation, Q7f pattern).
            for _ in range(n):
                fpt = psum_fi.tile([P, P], f32, tag="fill")
                nc.tensor.transpose(fpt, ident, ident)

        # ---- streaming blocks ----
        def sp_block(g):
            # one DMA (scalar/HWDGE#2): partition p holds rows p and 128+p
            spin = sp_pool.tile([P, 2, N], f32, tag="spin")
            nc.scalar.dma_start(spin, sp[g].rearrange("(c p) j -> p c j", c=2))
            # row masking (invalid node rows -> 0) on gpsimd
            nc.gpsimd.tensor_mul(
                spin[:, 0, :],
                spin[:, 0, :],
                xmT2[:, 2 * g : 2 * g + 1].to_broadcast([P, N]),
            )
            nc.gpsimd.tensor_mul(
                spin[:, 1, :],
                spin[:, 1, :],
                xmT2[:, 2 * g + 1 : 2 * g + 2].to_broadcast([P, N]),
            )
            # 4 PE transposes into one PSUM bank: [:, jc, :] = spT chunk jc
            pt = psum_tr.tile([P, 2, N], f32, tag="ptr")
            for jc in range(2):
                for ic in range(2):
                    nc.tensor.transpose(
                        pt[:, jc, ic * P : (ic + 1) * P],
                        spin[:, ic, jc * P : (jc + 1) * P],
                        ident,
                    )
            # one wide PSUM->SBUF copy on ACT
            sptw = spt_pool.tile([P, 2, N], f32, tag="sptw")
            nc.scalar.copy(sptw, pt[:, :, :])
            # fused colmax over both chunks -> CMall[:, 2g:2g+2], then mask
            cmv = CMall[:].rearrange("p (h c) -> p h c", c=2)[:, g, :]
            nc.vector.reduce_max(cmv, sptw, axis=AX.X)
            nc.vector.tensor_mul(cmv, cmv, xmT2[:, 2 * g : 2 * g + 2])
            # stage spT to DRAM (scalar/HWDGE#2) for indirect row gathers
            nc.scalar.dma_start(
                spt_dram[g].rearrange("(c p) i -> p c i", c=2), sptw
            )

        def attn_block(q):
            # 2 graphs per DMA, row-pair interleaved: 2KB descriptors
            g0 = GPER * q
            at = attn_pool.tile([P, GPER, NH, 2, N], f32r, tag="at")
            nc.sync.dma_start(
                at,
                attn[g0 : g0 + GPER]
                .rearrange("g h (p t) j -> p g h t j", t=2)
                .bitcast(f32r),
            )
            for gg in range(GPER):
                g = g0 + gg
                for t in range(2):
                    for h in range(NH):
                        idx = gg * 2 * NH + t * NH + h
                        nc.tensor.matmul(
                            scores_ps,
                            XBD[t][:, g * G : (g + 1) * G],
                            at[:, gg, h, t, :],
                            start=(q == 0 and idx == 0),
                            stop=(q == G // GPER - 1 and idx == GPER * 2 * NH - 1),
                        )

        # interleave so sp (and the FPS prep below) completes ~halfway
        NQ = G // GPER
        rowbase_i = cpool.tile([G, 1], i32)
        rowbase_f = cpool.tile([G, 1], f32)
        NMpre = cpool.tile([G, N], f32)
        SPB = G // (NQ // 2)  # sp blocks per first-half q
        for q in range(NQ // 2):
            for k in range(SPB):
                sp_block(SPB * q + k)
            attn_block(q)
            if q == 1:
                # needed by the FPS-prep gathers mid-stream
                nc.scalar.dma_start(rowbase_i, rowbi_d[:, :])
                nc.scalar.dma_start(rowbase_f, rowbf_d[:, :])
                nc.scalar.dma_start(NMpre, nmpre_d[:, :])

        # remaining constants (needed in the tail only)
        XM = cpool.tile([G, N], f32)
        nc.scalar.dma_start(XM, xm[:, :])
        iota_f = cpool.tile([G, N], f32)
        nc.scalar.dma_start(iota_f, iotaf_d[:, :])
        gb = cpool.tile([G, H], f32)
        nc.scalar.dma_start(gb, gb_d[:, :])
        bb = cpool.tile([G, H], f32)
        nc.scalar.dma_start(bb, bb_d[:, :])

        # ---- FPS prep (depends only on sp; runs during attn stream) ----
        Mtile = cpool.tile([P, G], f32)
        nc.vector.reduce_max(
            Mtile, CMall[:].rearrange("p (h c) -> p h c", c=2), axis=AX.X
        )
        pmt = psum_mi.tile([G, P], f32, tag="pmt")
        nc.tensor.transpose(pmt, Mtile, ident)
        MT = small.tile([G, P], f32)
        nc.vector.tensor_copy(MT, pmt)
        dmax = cpool.tile([G, 1], f32)
        nc.vector.reduce_max(dmax, MT, axis=AX.X)
        minspRaw = cpool.tile([G, N], f32)
        nc.gpsimd.indirect_dma_start(
            out=minspRaw,
            out_offset=None,
            in_=spt_flat,
            in_offset=IndirectOffsetOnAxis(ap=rowbase_i[:, :1], axis=0),
        )
        nc.vector.tensor_add(minspRaw, minspRaw, NMpre)
        xsum = cpool.tile([G, H], f32)
        nc.gpsimd.indirect_dma_start(
            out=xsum,
            out_offset=None,
            in_=x_flat,
            in_offset=IndirectOffsetOnAxis(ap=rowbase_i[:, :1], axis=0),
        )

        for q in range(NQ // 2, NQ - 1):
            attn_block(q)
            pe_filler(16)

        # last block split per graph: halves the MM work exposed after the
        # final attn byte lands (the stop=True matmul gates the whole tail)
        for gg in range(GPER):
            g = G - GPER + gg
            at = attn_pool.tile([P, 1, NH, 2, N], f32r, tag="at")
            nc.sync.dma_start(
                at,
                attn[g : g + 1]
                .rearrange("g h (p t) j -> p g h t j", t=2)
                .bitcast(f32r),
            )
            for t in range(2):
                for h in range(NH):
                    idx = t * NH + h
                    nc.tensor.matmul(
                        scores_ps,
                        XBD[t][:, g * G : (g + 1) * G],
                        at[:, 0, h, t, :],
                        start=False,
                        stop=(gg == GPER - 1 and idx == 2 * NH - 1),
                    )
            if gg == 0:
                pe_filler(8)

        # ---- tail: scores -> bonus -> FPS iterations ----
        # masked scores straight out of PSUM (fused copy+mask), then smax
        scoresAll = cpool.tile([G, N], f32)
        nc.vector.tensor_mul(scoresAll, scores_ps, XM)
        smax = small.tile([G, 1], f32)
        nc.vector.reduce_max(smax, scoresAll, axis=AX.X)
        inv_smax = small.tile([G, 1], f32)
        nc.vector.reciprocal(inv_smax, smax)
        # bonusM = scores * (0.1 * dmax / smax) + NM  (dmax-scaled domain;
        # carries the -BIG marks of invalid + already-selected nodes)
        sfac = small.tile([G, 1], f32)
        nc.vector.tensor_scalar(
            sfac, inv_smax, dmax[:, :1], 0.1, op0=OP.mult, op1=OP.mult
        )
        bonusM = cpool.tile([G, N], f32)
        nc.vector.tensor_scalar(bonusM, scoresAll, sfac[:, :1], None, op0=OP.mult)

        # cand = candAcc (min-chain with bonus folded in; marks live in bonusM
        # and propagate through the min since spcol+bonusM >= -BIG there)
        candAcc = cpool.tile([G, N], f32)
        nc.vector.tensor_add(candAcc, minspRaw, bonusM)
        for t in range(1, K):
            mx8 = small.tile([G, 8], f32, tag="mx8")
            nc.vector.max(out=mx8, in_=candAcc)
            ix8 = small.tile([G, 8], u32, tag="ix8")
            nc.vector.max_index(ix8, mx8, candAcc)
            offi = small.tile([G, 1], i32, tag="offi")
            nc.vector.tensor_scalar(
                offi, ix8[:, 0:1], rowbase_f[:, :1], None, op0=OP.add
            )
            if t < K - 1:
                spcol = fps.tile([G, N], f32, tag="spcol")
                nc.gpsimd.indirect_dma_start(
                    out=spcol,
                    out_offset=None,
                    in_=spt_flat,
                    in_offset=IndirectOffsetOnAxis(ap=offi[:, :1], axis=0),
                )
            # accumulate the selected x row in-flight
            nc.gpsimd.indirect_dma_start(
                out=xsum,
                out_offset=None,
                in_=x_flat,
                in_offset=IndirectOffsetOnAxis(ap=offi[:, :1], axis=0),
                compute_op=OP.add,
            )
            if t < K - 1:
                # mark the selected node in bonusM (overlaps the gather)
                idxf = small.tile([G, 1], f32, tag="idxf")
                nc.vector.tensor_copy(idxf, ix8[:, 0:1])
                ohB = fps.tile([G, N], f32, tag="ohB")
                nc.vector.tensor_scalar(
                    ohB, iota_f, idxf[:, :1], -BIG, op0=OP.is_equal, op1=OP.mult
                )
                nc.vector.tensor_add(bonusM, bonusM, ohB)
                spadj = fps.tile([G, N], f32, tag="spadj")
                nc.vector.tensor_add(spadj, spcol, bonusM)
                nc.vector.tensor_tensor(candAcc, candAcc, spadj, op=OP.min)

        # ---- LayerNorm on xsum (mean of K rows; eps scaled by K^2) ----
        st6 = small.tile([G, 6], f32)
        nc.vector.bn_stats(st6, xsum)
        mv = small.tile([G, 2], f32)
        nc.vector.bn_aggr(mv, st6)
        veps = small.tile([G, 1], f32)
        nc.vector.tensor_scalar(veps, mv[:, 1:2], float(K * K) * LN_EPS, None, op0=OP.add)
        std = small.tile([G, 1], f32)
        nc.scalar.sqrt(std, veps)
        rstd = small.tile([G, 1], f32)
        nc.vector.reciprocal(rstd, std)
        xn = cpool.tile([G, H], f32)
        nc.vector.tensor_scalar(
            xn, xsum, mv[:, 0:1], rstd[:, :1], op0=OP.subtract, op1=OP.mult
        )
        outt = cpool.tile([G, H], f32)
        nc.vector.tensor_mul(outt, xn, gb)
        nc.vector.tensor_add(outt, outt, bb)
        nc.sync.dma_start(out[:, :], outt)

    nc.compile()
    return nc


def core_inputs(core: int, x, attn, sp, xm, gamma, beta) -> dict:
    """Per-core input map incl. host-precomputed constants."""
    sl = slice(core * G, (core + 1) * G)
    xmc = np.ascontiguousarray(xm[sl])  # [G, N]
    lens = xmc.sum(axis=1).astype(np.int32)  # [G]

    pidx = np.arange(P)
    # chunk masks: node p (c=0) / node 128+p (c=1) valid, interleaved pairs
    xmt2 = np.zeros((P, 2 * G), dtype=np.float32)
    xmt2[:, 0::2] = (pidx[:, None] < lens[None, :]).astype(np.float32)
    xmt2[:, 1::2] = ((pidx[:, None] + P) < lens[None, :]).astype(np.float32)
    # parity masks: node 2p / 2p+1 valid, in block-diagonal layout
    xme = (2 * pidx[:, None] < lens[None, :]).astype(np.float32)
    xmo = ((2 * pidx[:, None] + 1) < lens[None, :]).astype(np.float32)
    xbde = np.zeros((P, G * G), dtype=np.float32)
    xbdo = np.zeros((P, G * G), dtype=np.float32)
    for g in range(G):
        xbde[:, g * G + g] = xme[:, g]
        xbdo[:, g * G + g] = xmo[:, g]
    # NM: 0 on available nodes, -BIG on invalid + node 0
    nmpre = (xmc - 1.0) * BIG
    nmpre[:, 0] = -BIG
    iotaf = np.broadcast_to(
        np.arange(N, dtype=np.float32)[None, :], (G, N)
    ).copy()
    rowb = (np.arange(G, dtype=np.int64) * N).reshape(G, 1)
    gbt = np.broadcast_to(gamma.reshape(1, H), (G, H)).copy()
    bbt = np.broadcast_to(beta.reshape(1, H), (G, H)).copy()
    return {
        "x": np.ascontiguousarray(x[sl]),
        "attn": np.ascontiguousarray(attn[sl]),
        "spatial_pos": np.ascontiguousarray(sp[sl]),
        "x_mask": xmc,
        "c_xbde": xbde,
        "c_xbdo": xbdo,
        "c_xmt2": xmt2,
        "c_nmpre": nmpre.astype(np.float32),
        "c_iotaf": iotaf,
        "c_rowbi": rowb.astype(np.int32),
        "c_rowbf": rowb.astype(np.float32),
        "c_gb": gbt.astype(np.float32),
        "c_bb": bbt.astype(np.float32),
    }


_NC_CACHE = None


def kernel(**inputs) -> np.ndarray:
    global _NC_CACHE, LAST_RESULT
    from concourse.bass_utils import run_bass_kernel_spmd

    x = np.ascontiguousarray(np.asarray(inputs["x"]), dtype=np.float32)
    attn = np.ascontiguousarray(np.asarray(inputs["attn"]), dtype=np.float32)
    sp = np.ascontiguousarray(np.asarray(inputs["spatial_pos"]), dtype=np.float32)
    xm = np.ascontiguousarray(np.asarray(inputs["x_mask"]), dtype=np.float32)
    gamma = np.asarray(inputs["gamma"], dtype=np.float32)
    beta = np.asarray(inputs["beta"], dtype=np.float32)

    if _NC_CACHE is None:
        _NC_CACHE = build_bass()
    nc = _NC_CACHE

    in_maps = [core_inputs(c, x, attn, sp, xm, gamma, beta) for c in range(NCORES)]

    res = run_bass_kernel_spmd(
        nc, in_maps, core_ids=list(range(NCORES)), trace=TRACE
    )
    LAST_RESULT = res
    return np.concatenate([r["out"] for r in res.results], axis=0)


# revision 46
# speedup vs baseline: 1.0368x; 1.0368x over previous
"""Trainium2 Bass kernel for nn_GraphPool (batched attentive FPS graph pooling).

Contract: kernel(**inputs) takes FULL inputs (B=128 graphs), shards the batch
dim across 8 NeuronCores (16 graphs each, pure data parallel), runs one SPMD
Bass program, and returns the FULL [128, 512] output.

Per-core algorithm (G=16 graphs, N=256 nodes, H=512, NH=8 heads, K=5):
  scores[g,j] = sum_{h, i<m} attn[g,h,i,j]  -> PE matmuls with block-diagonal
      0/1 mask weights (lhsT [128, 16], one nonzero column per graph) so all
      16 graphs accumulate into ONE psum tile [16, 256]; attn is DMA'd with
      row-pair interleave (i = 2p+t) giving 2KB descriptors, 2 graphs per DMA,
      masked via even/odd parity mask columns.
  sp: load row-pair chunks, row-mask on gpsimd, PE-transpose into a single
      PSUM bank [128, 512], one wide ACT copy out, fused 3D colmax -> dmax;
      spT staged to DRAM for the FPS column gathers (as row gathers).
  FPS in the dmax-scaled domain (cand' = cand*dmax): candAcc = min-chain with
      the bonus folded in; -BIG marks for invalid/selected nodes live in
      bonusM and ride every min-update; per iteration: max/max_index ->
      indirect gather of the selected sp column -> min.
  pool: x rows gathered with the same offsets, accumulated in-flight via
      DMA compute_op=add; LayerNorm via bn_stats/bn_aggr (eps scaled by K^2).

All mask/iota constants are precomputed on the host and passed as extra
inputs — deriving them on-chip serialized the first ~35us of the kernel.
(tensor_tensor_reduce and indirect compute_op min/max are avoided: rejected
or exec-unit-wedging on this HW; compute_op=add is fine.)
"""

import os
import sys
from contextlib import ExitStack

for _p in ("/opt/trn_rl_repo", "/root/.axon_site/_ro/trn_rl_repo"):
    if os.path.isdir(_p) and _p not in sys.path:
        sys.path.append(_p)

import numpy as np

import concourse.mybir as mybir
from concourse.bass import Bass, IndirectOffsetOnAxis
from concourse.bacc import Bacc
from concourse.masks import make_identity
from concourse.tile import TileContext

B, N, H, NH, K = 128, 256, 512, 8, 5
NCORES = 8
G = B // NCORES  # graphs per core
P = 128
LN_EPS = 1e-5
BIG = 1.0e30  # unavailable-node mark

f32 = mybir.dt.float32
f32r = mybir.dt.float32r
i32 = mybir.dt.int32
u32 = mybir.dt.uint32
AX = mybir.AxisListType
OP = mybir.AluOpType

TRACE = False
LAST_RESULT = None
GPER = 2  # graphs per attn DMA


def build_bass() -> Bass:
    nc = Bacc()
    x = nc.dram_tensor("x", [G, N, H], f32, kind="ExternalInput")
    attn = nc.dram_tensor("attn", [G, NH, N, N], f32, kind="ExternalInput")
    sp = nc.dram_tensor("spatial_pos", [G, N, N], f32, kind="ExternalInput")
    xm = nc.dram_tensor("x_mask", [G, N], f32, kind="ExternalInput")
    # host-precomputed constants (see core_inputs)
    xbde_d = nc.dram_tensor("c_xbde", [P, G * G], f32r, kind="ExternalInput")
    xbdo_d = nc.dram_tensor("c_xbdo", [P, G * G], f32r, kind="ExternalInput")
    xmt2_d = nc.dram_tensor("c_xmt2", [P, 2 * G], f32, kind="ExternalInput")
    nmpre_d = nc.dram_tensor("c_nmpre", [G, N], f32, kind="ExternalInput")
    iotaf_d = nc.dram_tensor("c_iotaf", [G, N], f32, kind="ExternalInput")
    rowbi_d = nc.dram_tensor("c_rowbi", [G, 1], i32, kind="ExternalInput")
    rowbf_d = nc.dram_tensor("c_rowbf", [G, 1], f32, kind="ExternalInput")
    gb_d = nc.dram_tensor("c_gb", [G, H], f32, kind="ExternalInput")
    bb_d = nc.dram_tensor("c_bb", [G, H], f32, kind="ExternalInput")
    out = nc.dram_tensor("out", [G, H], f32, kind="ExternalOutput")
    spt_dram = nc.dram_tensor("spt_scratch", [G, N, N], f32, kind="Internal")

    x_flat = x[:].rearrange("g n h -> (g n) h")
    spt_flat = spt_dram[:].rearrange("g n j -> (g n) j")

    with TileContext(nc) as tc, ExitStack() as ctx:
        cpool = ctx.enter_context(tc.tile_pool(name="cpool", bufs=1))
        small = ctx.enter_context(tc.tile_pool(name="small", bufs=2))
        fps = ctx.enter_context(tc.tile_pool(name="fps", bufs=2))
        attn_pool = ctx.enter_context(tc.tile_pool(name="attn_pool", bufs=3))
        sp_pool = ctx.enter_context(tc.tile_pool(name="sp_pool", bufs=6))
        spt_pool = ctx.enter_context(tc.tile_pool(name="spt_pool", bufs=4))
        psum_sc = ctx.enter_context(tc.tile_pool(name="psum_sc", bufs=1, space="PSUM"))
        psum_tr = ctx.enter_context(tc.tile_pool(name="psum_tr", bufs=4, space="PSUM"))
        psum_mi = ctx.enter_context(tc.tile_pool(name="psum_mi", bufs=1, space="PSUM"))
        psum_fi = ctx.enter_context(tc.tile_pool(name="psum_fi", bufs=2, space="PSUM"))

        # ---- constant loads (critical ones first on sync, rest on scalar) ----
        XBDe = cpool.tile([P, G * G], f32r)
        nc.sync.dma_start(XBDe, xbde_d[:, :])
        XBDo = cpool.tile([P, G * G], f32r)
        nc.sync.dma_start(XBDo, xbdo_d[:, :])
        XBD = (XBDe, XBDo)

        xmT2 = cpool.tile([P, 2 * G], f32)
        nc.scalar.dma_start(xmT2, xmt2_d[:, :])

        ident = cpool.tile([P, P], f32)
        make_identity(nc, ident)

        CMall = cpool.tile([P, 2 * G], f32)
        scores_ps = psum_sc.tile([G, N], f32)

        def pe_filler(n):
            # Dummy transposes (PE-local, never read): keep the PE activity
            # monitor's clock gate at 8/8 through DMA waits so real matmuls
            # run at 2.4 GHz instead of 1.2 (HAM oscillation, Q7f pattern).
            for _ in range(n):
                fpt = psum_fi.tile([P, P], f32, tag="fill")
                nc.tensor.transpose(fpt, ident, ident)

        # ---- streaming blocks ----
        def sp_block(g):
            # one DMA (scalar/HWDGE#2): partition p holds rows p and 128+p
            spin = sp_pool.tile([P, 2, N], f32, tag="spin")
            nc.scalar.dma_start(spin, sp[g].rearrange("(c p) j -> p c j", c=2))
            # row masking (invalid node rows -> 0) on gpsimd
            nc.gpsimd.tensor_mul(
                spin[:, 0, :],
                spin[:, 0, :],
                xmT2[:, 2 * g : 2 * g + 1].to_broadcast([P, N]),
            )
            nc.gpsimd.tensor_mul(
                spin[:, 1, :],
                spin[:, 1, :],
                xmT2[:, 2 * g + 1 : 2 * g + 2].to_broadcast([P, N]),
            )
            # 4 PE transposes into one PSUM bank: [:, jc, :] = spT chunk jc
            pt = psum_tr.tile([P, 2, N], f32, tag="ptr")
            for jc in range(2):
                for ic in range(2):
                    nc.tensor.transpose(
                        pt[:, jc, ic * P : (ic + 1) * P],
                        spin[:, ic, jc * P : (jc + 1) * P],
                        ident,
                    )
            # one wide PSUM->SBUF copy on ACT
            sptw = spt_pool.tile([P, 2, N], f32, tag="sptw")
            nc.scalar.copy(sptw, pt[:, :, :])
            # fused colmax over both chunks -> CMall[:, 2g:2g+2], then mask
            cmv = CMall[:].rearrange("p (h c) -> p h c", c=2)[:, g, :]
            nc.vector.reduce_max(cmv, sptw, axis=AX.X)
            nc.vector.tensor_mul(cmv, cmv, xmT2[:, 2 * g : 2 * g + 2])
            # stage spT to DRAM (scalar/HWDGE#2) for indirect row gathers
            nc.scalar.dma_start(
                spt_dram[g].rearrange("(c p) i -> p c i", c=2), sptw
            )

        def attn_block(q):
            # 2 graphs per DMA, row-pair interleaved: 2KB descriptors
            g0 = GPER * q
            at = attn_pool.tile([P, GPER, NH, 2, N], f32r, tag="at")
            nc.sync.dma_start(
                at,
                attn[g0 : g0 + GPER]
                .rearrange("g h (p t) j -> p g h t j", t=2)
                .bitcast(f32r),
            )
            for gg in range(GPER):
                g = g0 + gg
                for t in range(2):
                    for h in range(NH):
                        idx = gg * 2 * NH + t * NH + h
                        nc.tensor.matmul(
                            scores_ps,
                            XBD[t][:, g * G : (g + 1) * G],
                            at[:, gg, h, t, :],
                            start=(q == 0 and idx == 0),
                            stop=(q == G // GPER - 1 and idx == GPER * 2 * NH - 1),
                        )

        # interleave so sp (and the FPS prep below) completes ~halfway
        NQ = G // GPER
        rowbase_i = cpool.tile([G, 1], i32)
        rowbase_f = cpool.tile([G, 1], f32)
        SPB = G // (NQ // 2)  # sp blocks per first-half q
        for q in range(NQ // 2):
            for k in range(SPB):
                sp_block(SPB * q + k)
            attn_block(q)
            if q == 1:
                # needed by the FPS-prep gathers mid-stream
                nc.scalar.dma_start(rowbase_i, rowbi_d[:, :])
                nc.scalar.dma_start(rowbase_f, rowbf_d[:, :])

        # remaining constants (needed in the tail only)
        XM = cpool.tile([G, N], f32)
        nc.scalar.dma_start(XM, xm[:, :])
        NMpre = cpool.tile([G, N], f32)
        nc.scalar.dma_start(NMpre, nmpre_d[:, :])
        iota_f = cpool.tile([G, N], f32)
        nc.scalar.dma_start(iota_f, iotaf_d[:, :])
        gb = cpool.tile([G, H], f32)
        nc.scalar.dma_start(gb, gb_d[:, :])
        bb = cpool.tile([G, H], f32)
        nc.scalar.dma_start(bb, bb_d[:, :])

        # ---- FPS prep (depends only on sp; runs during attn stream) ----
        Mtile = cpool.tile([P, G], f32)
        nc.vector.reduce_max(
            Mtile, CMall[:].rearrange("p (h c) -> p h c", c=2), axis=AX.X
        )
        pmt = psum_mi.tile([G, P], f32, tag="pmt")
        nc.tensor.transpose(pmt, Mtile, ident)
        MT = small.tile([G, P], f32)
        nc.vector.tensor_copy(MT, pmt)
        dmax = cpool.tile([G, 1], f32)
        nc.vector.reduce_max(dmax, MT, axis=AX.X)
        minspRaw = cpool.tile([G, N], f32)
        nc.gpsimd.indirect_dma_start(
            out=minspRaw,
            out_offset=None,
            in_=spt_flat,
            in_offset=IndirectOffsetOnAxis(ap=rowbase_i[:, :1], axis=0),
        )
        xsum = cpool.tile([G, H], f32)
        nc.gpsimd.indirect_dma_start(
            out=xsum,
            out_offset=None,
            in_=x_flat,
            in_offset=IndirectOffsetOnAxis(ap=rowbase_i[:, :1], axis=0),
        )

        for q in range(NQ // 2, NQ - 1):
            attn_block(q)
            pe_filler(16)

        # last block split per graph: halves the MM work exposed after the
        # final attn byte lands (the stop=True matmul gates the whole tail)
        for gg in range(GPER):
            g = G - GPER + gg
            at = attn_pool.tile([P, 1, NH, 2, N], f32r, tag="at")
            nc.sync.dma_start(
                at,
                attn[g : g + 1]
                .rearrange("g h (p t) j -> p g h t j", t=2)
                .bitcast(f32r),
            )
            for t in range(2):
                for h in range(NH):
                    idx = t * NH + h
                    nc.tensor.matmul(
                        scores_ps,
                        XBD[t][:, g * G : (g + 1) * G],
                        at[:, 0, h, t, :],
                        start=False,
                        stop=(gg == GPER - 1 and idx == 2 * NH - 1),
                    )
            if gg == 0:
                pe_filler(8)

        # ---- tail: scores -> bonus -> FPS iterations ----
        # masked scores straight out of PSUM (fused copy+mask), then smax
        scoresAll = cpool.tile([G, N], f32)
        nc.vector.tensor_mul(scoresAll, scores_ps, XM)
        smax = small.tile([G, 1], f32)
        nc.vector.reduce_max(smax, scoresAll, axis=AX.X)
        inv_smax = small.tile([G, 1], f32)
        nc.vector.reciprocal(inv_smax, smax)
        # bonusM = scores * (0.1 * dmax / smax) + NM  (dmax-scaled domain;
        # carries the -BIG marks of invalid + already-selected nodes)
        sfac = small.tile([G, 1], f32)
        nc.vector.tensor_scalar(
            sfac, inv_smax, dmax[:, :1], 0.1, op0=OP.mult, op1=OP.mult
        )
        bonusM = cpool.tile([G, N], f32)
        nc.vector.tensor_scalar(bonusM, scoresAll, sfac[:, :1], None, op0=OP.mult)
        nc.vector.tensor_add(bonusM, bonusM, NMpre)

        # cand = candAcc (min-chain with bonus folded in; marks live in bonusM
        # and propagate through the min since spcol+bonusM >= -BIG there)
        candAcc = cpool.tile([G, N], f32)
        nc.vector.tensor_add(candAcc, minspRaw, bonusM)
        for t in range(1, K):
            mx8 = small.tile([G, 8], f32, tag="mx8")
            nc.vector.max(out=mx8, in_=candAcc)
            ix8 = small.tile([G, 8], u32, tag="ix8")
            nc.vector.max_index(ix8, mx8, candAcc)
            offi = small.tile([G, 1], i32, tag="offi")
            nc.vector.tensor_scalar(
                offi, ix8[:, 0:1], rowbase_f[:, :1], None, op0=OP.add
            )
            if t < K - 1:
                spcol = fps.tile([G, N], f32, tag="spcol")
                nc.gpsimd.indirect_dma_start(
                    out=spcol,
                    out_offset=None,
                    in_=spt_flat,
                    in_offset=IndirectOffsetOnAxis(ap=offi[:, :1], axis=0),
                )
            # accumulate the selected x row in-flight
            nc.gpsimd.indirect_dma_start(
                out=xsum,
                out_offset=None,
                in_=x_flat,
                in_offset=IndirectOffsetOnAxis(ap=offi[:, :1], axis=0),
                compute_op=OP.add,
            )
            if t < K - 1:
                # mark the selected node in bonusM (overlaps the gather)
                idxf = small.tile([G, 1], f32, tag="idxf")
                nc.vector.tensor_copy(idxf, ix8[:, 0:1])
                ohB = fps.tile([G, N], f32, tag="ohB")
                nc.vector.tensor_scalar(
                    ohB, iota_f, idxf[:, :1], -BIG, op0=OP.is_equal, op1=OP.mult
                )
                nc.vector.tensor_add(bonusM, bonusM, ohB)
                spadj = fps.tile([G, N], f32, tag="spadj")
                nc.vector.tensor_add(spadj, spcol, bonusM)
                nc.vector.tensor_tensor(candAcc, candAcc, spadj, op=OP.min)

        # ---- LayerNorm on xsum (mean of K rows; eps scaled by K^2) ----
        st6 = small.tile([G, 6], f32)
        nc.vector.bn_stats(st6, xsum)
        mv = small.tile([G, 2], f32)
        nc.vector.bn_aggr(mv, st6)
        veps = small.tile([G, 1], f32)
        nc.vector.tensor_scalar(veps, mv[:, 1:2], float(K * K) * LN_EPS, None, op0=OP.add)
        std = small.tile([G, 1], f32)
        nc.scalar.sqrt(std, veps)
        rstd = small.tile([G, 1], f32)
        nc.vector.reciprocal(rstd, std)
        xn = cpool.tile([G, H], f32)
        nc.vector.tensor_scalar(
            xn, xsum, mv[:, 0:1], rstd[:, :1], op0=OP.subtract, op1=OP.mult
        )
        outt = cpool.tile([G, H], f32)
        nc.vector.tensor_mul(outt, xn, gb)
        nc.vector.tensor_add(outt, outt, bb)
        nc.sync.dma_start(out[:, :], outt)

    nc.compile()
    return nc


def core_inputs(core: int, x, attn, sp, xm, gamma, beta) -> dict:
    """Per-core input map incl. host-precomputed constants."""
    sl = slice(core * G, (core + 1) * G)
    xmc = np.ascontiguousarray(xm[sl])  # [G, N]
    lens = xmc.sum(axis=1).astype(np.int32)  # [G]

    pidx = np.arange(P)
    # chunk masks: node p (c=0) / node 128+p (c=1) valid, interleaved pairs
    xmt2 = np.zeros((P, 2 * G), dtype=np.float32)
    xmt2[:, 0::2] = (pidx[:, None] < lens[None, :]).astype(np.float32)
    xmt2[:, 1::2] = ((pidx[:, None] + P) < lens[None, :]).astype(np.float32)
    # parity masks: node 2p / 2p+1 valid, in block-diagonal layout
    xme = (2 * pidx[:, None] < lens[None, :]).astype(np.float32)
    xmo = ((2 * pidx[:, None] + 1) < lens[None, :]).astype(np.float32)
    xbde = np.zeros((P, G * G), dtype=np.float32)
    xbdo = np.zeros((P, G * G), dtype=np.float32)
    for g in range(G):
        xbde[:, g * G + g] = xme[:, g]
        xbdo[:, g * G + g] = xmo[:, g]
    # NM: 0 on available nodes, -BIG on invalid + node 0
    nmpre = (xmc - 1.0) * BIG
    nmpre[:, 0] = -BIG
    iotaf = np.broadcast_to(
        np.arange(N, dtype=np.float32)[None, :], (G, N)
    ).copy()
    rowb = (np.arange(G, dtype=np.int64) * N).reshape(G, 1)
    gbt = np.broadcast_to(gamma.reshape(1, H), (G, H)).copy()
    bbt = np.broadcast_to(beta.reshape(1, H), (G, H)).copy()
    return {
        "x": np.ascontiguousarray(x[sl]),
        "attn": np.ascontiguousarray(attn[sl]),
        "spatial_pos": np.ascontiguousarray(sp[sl]),
        "x_mask": xmc,
        "c_xbde": xbde,
        "c_xbdo": xbdo,
        "c_xmt2": xmt2,
        "c_nmpre": nmpre.astype(np.float32),
        "c_iotaf": iotaf,
        "c_rowbi": rowb.astype(np.int32),
        "c_rowbf": rowb.astype(np.float32),
        "c_gb": gbt.astype(np.float32),
        "c_bb": bbt.astype(np.float32),
    }


_NC_CACHE = None


def kernel(**inputs) -> np.ndarray:
    global _NC_CACHE, LAST_RESULT
    from concourse.bass_utils import run_bass_kernel_spmd

    x = np.ascontiguousarray(np.asarray(inputs["x"]), dtype=np.float32)
    attn = np.ascontiguousarray(np.asarray(inputs["attn"]), dtype=np.float32)
    sp = np.ascontiguousarray(np.asarray(inputs["spatial_pos"]), dtype=np.float32)
    xm = np.ascontiguousarray(np.asarray(inputs["x_mask"]), dtype=np.float32)
    gamma = np.asarray(inputs["gamma"], dtype=np.float32)
    beta = np.asarray(inputs["beta"], dtype=np.float32)

    if _NC_CACHE is None:
        _NC_CACHE = build_bass()
    nc = _NC_CACHE

    in_maps = [core_inputs(c, x, attn, sp, xm, gamma, beta) for c in range(NCORES)]

    res = run_bass_kernel_spmd(
        nc, in_maps, core_ids=list(range(NCORES)), trace=TRACE
    )
    LAST_RESULT = res
    return np.concatenate([r["out"] for r in res.results], axis=0)


# revision 48
# speedup vs baseline: 1.0647x; 1.0270x over previous
"""Trainium2 Bass kernel for nn_GraphPool (batched attentive FPS graph pooling).

Contract: kernel(**inputs) takes FULL inputs (B=128 graphs), shards the batch
dim across 8 NeuronCores (16 graphs each, pure data parallel), runs one SPMD
Bass program, and returns the FULL [128, 512] output.

Per-core algorithm (G=16 graphs, N=256 nodes, H=512, NH=8 heads, K=5):
  scores[g,j] = sum_{h, i<m} attn[g,h,i,j]  -> PE matmuls with block-diagonal
      0/1 mask weights (lhsT [128, 16], one nonzero column per graph) so all
      16 graphs accumulate into ONE psum tile [16, 256]; attn is DMA'd with
      row-pair interleave (i = 2p+t) giving 2KB descriptors, 2 graphs per DMA,
      masked via even/odd parity mask columns.
  sp: load row-pair chunks, row-mask on gpsimd, PE-transpose into a single
      PSUM bank [128, 512], one wide ACT copy out, fused 3D colmax -> dmax;
      spT staged to DRAM for the FPS column gathers (as row gathers).
  FPS in the dmax-scaled domain (cand' = cand*dmax): candAcc = min-chain with
      the bonus folded in; -BIG marks for invalid/selected nodes live in
      bonusM and ride every min-update; per iteration: max/max_index ->
      indirect gather of the selected sp column -> min.
  pool: x rows gathered with the same offsets, accumulated in-flight via
      DMA compute_op=add; LayerNorm via bn_stats/bn_aggr (eps scaled by K^2).

All mask/iota constants are precomputed on the host and passed as extra
inputs — deriving them on-chip serialized the first ~35us of the kernel.
(tensor_tensor_reduce and indirect compute_op min/max are avoided: rejected
or exec-unit-wedging on this HW; compute_op=add is fine.)
"""

import os
import sys
from contextlib import ExitStack

for _p in ("/opt/trn_rl_repo", "/root/.axon_site/_ro/trn_rl_repo"):
    if os.path.isdir(_p) and _p not in sys.path:
        sys.path.append(_p)

import numpy as np

import concourse.mybir as mybir
from concourse.bass import Bass, IndirectOffsetOnAxis
from concourse.bacc import Bacc
from concourse.masks import make_identity
from concourse.tile import TileContext

B, N, H, NH, K = 128, 256, 512, 8, 5
NCORES = 8
G = B // NCORES  # graphs per core
P = 128
LN_EPS = 1e-5
BIG = 1.0e30  # unavailable-node mark

f32 = mybir.dt.float32
f32r = mybir.dt.float32r
i32 = mybir.dt.int32
u32 = mybir.dt.uint32
AX = mybir.AxisListType
OP = mybir.AluOpType

TRACE = False
LAST_RESULT = None
GPER = 2  # graphs per attn DMA


def build_bass(trivial_affine: bool = False) -> Bass:
    nc = Bacc()
    x = nc.dram_tensor("x", [G, N, H], f32, kind="ExternalInput")
    attn = nc.dram_tensor("attn", [G, NH, N, N], f32, kind="ExternalInput")
    sp = nc.dram_tensor("spatial_pos", [G, N, N], f32, kind="ExternalInput")
    xm = nc.dram_tensor("x_mask", [G, N], f32, kind="ExternalInput")
    # host-precomputed constants (see core_inputs)
    xbde_d = nc.dram_tensor("c_xbde", [P, G * G], f32r, kind="ExternalInput")
    xbdo_d = nc.dram_tensor("c_xbdo", [P, G * G], f32r, kind="ExternalInput")
    xmt2_d = nc.dram_tensor("c_xmt2", [P, 2 * G], f32, kind="ExternalInput")
    nmpre_d = nc.dram_tensor("c_nmpre", [G, N], f32, kind="ExternalInput")
    iotaf_d = nc.dram_tensor("c_iotaf", [G, N], f32, kind="ExternalInput")
    rowbi_d = nc.dram_tensor("c_rowbi", [G, 1], i32, kind="ExternalInput")
    rowbf_d = nc.dram_tensor("c_rowbf", [G, 1], f32, kind="ExternalInput")
    if not trivial_affine:
        gb_d = nc.dram_tensor("c_gb", [G, H], f32, kind="ExternalInput")
        bb_d = nc.dram_tensor("c_bb", [G, H], f32, kind="ExternalInput")
    out = nc.dram_tensor("out", [G, H], f32, kind="ExternalOutput")
    spt_dram = nc.dram_tensor("spt_scratch", [G, N, N], f32, kind="Internal")

    x_flat = x[:].rearrange("g n h -> (g n) h")
    spt_flat = spt_dram[:].rearrange("g n j -> (g n) j")

    with TileContext(nc) as tc, ExitStack() as ctx:
        cpool = ctx.enter_context(tc.tile_pool(name="cpool", bufs=1))
        small = ctx.enter_context(tc.tile_pool(name="small", bufs=2))
        fps = ctx.enter_context(tc.tile_pool(name="fps", bufs=2))
        attn_pool = ctx.enter_context(tc.tile_pool(name="attn_pool", bufs=3))
        sp_pool = ctx.enter_context(tc.tile_pool(name="sp_pool", bufs=6))
        spt_pool = ctx.enter_context(tc.tile_pool(name="spt_pool", bufs=4))
        psum_sc = ctx.enter_context(tc.tile_pool(name="psum_sc", bufs=1, space="PSUM"))
        psum_tr = ctx.enter_context(tc.tile_pool(name="psum_tr", bufs=4, space="PSUM"))
        psum_mi = ctx.enter_context(tc.tile_pool(name="psum_mi", bufs=1, space="PSUM"))
        psum_fi = ctx.enter_context(tc.tile_pool(name="psum_fi", bufs=2, space="PSUM"))

        # ---- constant loads (critical ones first on sync, rest on scalar) ----
        XBDe = cpool.tile([P, G * G], f32r)
        nc.sync.dma_start(XBDe, xbde_d[:, :])
        XBDo = cpool.tile([P, G * G], f32r)
        nc.sync.dma_start(XBDo, xbdo_d[:, :])
        XBD = (XBDe, XBDo)

        xmT2 = cpool.tile([P, 2 * G], f32)
        nc.scalar.dma_start(xmT2, xmt2_d[:, :])

        ident = cpool.tile([P, P], f32)
        make_identity(nc, ident)

        CMall = cpool.tile([P, 2 * G], f32)
        scores_ps = psum_sc.tile([G, N], f32)

        def pe_filler(n):
            # Dummy transposes (PE-local, never read): keep the PE activity
            # monitor's clock gate at 8/8 through DMA waits so real matmuls
            # run at 2.4 GHz instead of 1.2 (HAM oscillation, Q7f pattern).
            for _ in range(n):
                fpt = psum_fi.tile([P, P], f32, tag="fill")
                nc.tensor.transpose(fpt, ident, ident)

        # ---- streaming blocks ----
        def sp_block(g):
            # one DMA (scalar/HWDGE#2): partition p holds rows p and 128+p
            spin = sp_pool.tile([P, 2, N], f32, tag="spin")
            nc.scalar.dma_start(spin, sp[g].rearrange("(c p) j -> p c j", c=2))
            # row masking (invalid node rows -> 0) on gpsimd
            nc.gpsimd.tensor_mul(
                spin[:, 0, :],
                spin[:, 0, :],
                xmT2[:, 2 * g : 2 * g + 1].to_broadcast([P, N]),
            )
            nc.gpsimd.tensor_mul(
                spin[:, 1, :],
                spin[:, 1, :],
                xmT2[:, 2 * g + 1 : 2 * g + 2].to_broadcast([P, N]),
            )
            # 4 PE transposes into one PSUM bank: [:, jc, :] = spT chunk jc
            pt = psum_tr.tile([P, 2, N], f32, tag="ptr")
            for jc in range(2):
                for ic in range(2):
                    nc.tensor.transpose(
                        pt[:, jc, ic * P : (ic + 1) * P],
                        spin[:, ic, jc * P : (jc + 1) * P],
                        ident,
                    )
            # one wide PSUM->SBUF copy on ACT
            sptw = spt_pool.tile([P, 2, N], f32, tag="sptw")
            nc.scalar.copy(sptw, pt[:, :, :])
            # fused colmax over both chunks -> CMall[:, 2g:2g+2], then mask
            cmv = CMall[:].rearrange("p (h c) -> p h c", c=2)[:, g, :]
            nc.vector.reduce_max(cmv, sptw, axis=AX.X)
            nc.vector.tensor_mul(cmv, cmv, xmT2[:, 2 * g : 2 * g + 2])
            # stage spT to DRAM (scalar/HWDGE#2) for indirect row gathers
            nc.scalar.dma_start(
                spt_dram[g].rearrange("(c p) i -> p c i", c=2), sptw
            )

        def attn_block(q):
            # 2 graphs per DMA, row-pair interleaved: 2KB descriptors
            g0 = GPER * q
            at = attn_pool.tile([P, GPER, NH, 2, N], f32r, tag="at")
            nc.sync.dma_start(
                at,
                attn[g0 : g0 + GPER]
                .rearrange("g h (p t) j -> p g h t j", t=2)
                .bitcast(f32r),
            )
            for gg in range(GPER):
                g = g0 + gg
                for t in range(2):
                    for h in range(NH):
                        idx = gg * 2 * NH + t * NH + h
                        nc.tensor.matmul(
                            scores_ps,
                            XBD[t][:, g * G : (g + 1) * G],
                            at[:, gg, h, t, :],
                            start=(q == 0 and idx == 0),
                            stop=(q == G // GPER - 1 and idx == GPER * 2 * NH - 1),
                        )

        # interleave so sp (and the FPS prep below) completes ~halfway
        NQ = G // GPER
        rowbase_i = cpool.tile([G, 1], i32)
        rowbase_f = cpool.tile([G, 1], f32)
        NMpre = cpool.tile([G, N], f32)
        SPB = G // (NQ // 2)  # sp blocks per first-half q
        for q in range(NQ // 2):
            for k in range(SPB):
                sp_block(SPB * q + k)
            attn_block(q)
            if q == 1:
                # needed by the FPS-prep gathers mid-stream
                nc.scalar.dma_start(rowbase_i, rowbi_d[:, :])
                nc.scalar.dma_start(rowbase_f, rowbf_d[:, :])
                nc.scalar.dma_start(NMpre, nmpre_d[:, :])

        # remaining constants (needed in the tail only)
        XM = cpool.tile([G, N], f32)
        nc.scalar.dma_start(XM, xm[:, :])
        iota_f = cpool.tile([G, N], f32)
        nc.scalar.dma_start(iota_f, iotaf_d[:, :])
        if not trivial_affine:
            gb = cpool.tile([G, H], f32)
            nc.scalar.dma_start(gb, gb_d[:, :])
            bb = cpool.tile([G, H], f32)
            nc.scalar.dma_start(bb, bb_d[:, :])

        # ---- FPS prep (depends only on sp; runs during attn stream) ----
        Mtile = cpool.tile([P, G], f32)
        nc.vector.reduce_max(
            Mtile, CMall[:].rearrange("p (h c) -> p h c", c=2), axis=AX.X
        )
        pmt = psum_mi.tile([G, P], f32, tag="pmt")
        nc.tensor.transpose(pmt, Mtile, ident)
        MT = small.tile([G, P], f32)
        nc.vector.tensor_copy(MT, pmt)
        dmax = cpool.tile([G, 1], f32)
        nc.vector.reduce_max(dmax, MT, axis=AX.X)
        minspRaw = cpool.tile([G, N], f32)
        nc.gpsimd.indirect_dma_start(
            out=minspRaw,
            out_offset=None,
            in_=spt_flat,
            in_offset=IndirectOffsetOnAxis(ap=rowbase_i[:, :1], axis=0),
        )
        nc.vector.tensor_add(minspRaw, minspRaw, NMpre)
        xsum = cpool.tile([G, H], f32)
        nc.gpsimd.indirect_dma_start(
            out=xsum,
            out_offset=None,
            in_=x_flat,
            in_offset=IndirectOffsetOnAxis(ap=rowbase_i[:, :1], axis=0),
        )

        for q in range(NQ // 2, NQ - 1):
            attn_block(q)
            pe_filler(16)

        # last block split per graph: halves the MM work exposed after the
        # final attn byte lands (the stop=True matmul gates the whole tail)
        for gg in range(GPER):
            g = G - GPER + gg
            at = attn_pool.tile([P, 1, NH, 2, N], f32r, tag="at")
            nc.sync.dma_start(
                at,
                attn[g : g + 1]
                .rearrange("g h (p t) j -> p g h t j", t=2)
                .bitcast(f32r),
            )
            for t in range(2):
                for h in range(NH):
                    idx = t * NH + h
                    nc.tensor.matmul(
                        scores_ps,
                        XBD[t][:, g * G : (g + 1) * G],
                        at[:, 0, h, t, :],
                        start=False,
                        stop=(gg == GPER - 1 and idx == 2 * NH - 1),
                    )
            if gg == 0:
                pe_filler(8)

        # ---- tail: scores -> bonus -> FPS iterations ----
        # masked scores straight out of PSUM (fused copy+mask), then smax
        scoresAll = cpool.tile([G, N], f32)
        nc.vector.tensor_mul(scoresAll, scores_ps, XM)
        smax = small.tile([G, 1], f32)
        nc.vector.reduce_max(smax, scoresAll, axis=AX.X)
        inv_smax = small.tile([G, 1], f32)
        nc.vector.reciprocal(inv_smax, smax)
        # bonusM = scores * (0.1 * dmax / smax) + NM  (dmax-scaled domain;
        # carries the -BIG marks of invalid + already-selected nodes)
        sfac = small.tile([G, 1], f32)
        nc.vector.tensor_scalar(
            sfac, inv_smax, dmax[:, :1], 0.1, op0=OP.mult, op1=OP.mult
        )
        bonusM = cpool.tile([G, N], f32)
        nc.vector.tensor_scalar(bonusM, scoresAll, sfac[:, :1], None, op0=OP.mult)

        # cand = candAcc (min-chain with bonus folded in; marks live in bonusM
        # and propagate through the min since spcol+bonusM >= -BIG there)
        candAcc = cpool.tile([G, N], f32)
        nc.vector.tensor_add(candAcc, minspRaw, bonusM)
        for t in range(1, K):
            mx8 = small.tile([G, 8], f32, tag="mx8")
            nc.vector.max(out=mx8, in_=candAcc)
            ix8 = small.tile([G, 8], u32, tag="ix8")
            nc.vector.max_index(ix8, mx8, candAcc)
            offi = small.tile([G, 1], i32, tag="offi")
            nc.vector.tensor_scalar(
                offi, ix8[:, 0:1], rowbase_f[:, :1], None, op0=OP.add
            )
            if t < K - 1:
                spcol = fps.tile([G, N], f32, tag="spcol")
                nc.gpsimd.indirect_dma_start(
                    out=spcol,
                    out_offset=None,
                    in_=spt_flat,
                    in_offset=IndirectOffsetOnAxis(ap=offi[:, :1], axis=0),
                )
            # accumulate the selected x row in-flight
            nc.gpsimd.indirect_dma_start(
                out=xsum,
                out_offset=None,
                in_=x_flat,
                in_offset=IndirectOffsetOnAxis(ap=offi[:, :1], axis=0),
                compute_op=OP.add,
            )
            if t < K - 1:
                # mark the selected node in bonusM (overlaps the gather)
                idxf = small.tile([G, 1], f32, tag="idxf")
                nc.vector.tensor_copy(idxf, ix8[:, 0:1])
                ohB = fps.tile([G, N], f32, tag="ohB")
                nc.vector.tensor_scalar(
                    ohB, iota_f, idxf[:, :1], -BIG, op0=OP.is_equal, op1=OP.mult
                )
                nc.vector.tensor_add(bonusM, bonusM, ohB)
                spadj = fps.tile([G, N], f32, tag="spadj")
                nc.vector.tensor_add(spadj, spcol, bonusM)
                nc.vector.tensor_tensor(candAcc, candAcc, spadj, op=OP.min)

        # ---- LayerNorm on xsum (mean of K rows; eps scaled by K^2) ----
        st6 = small.tile([G, 6], f32)
        nc.vector.bn_stats(st6, xsum)
        mv = small.tile([G, 2], f32)
        nc.vector.bn_aggr(mv, st6)
        veps = small.tile([G, 1], f32)
        nc.vector.tensor_scalar(veps, mv[:, 1:2], float(K * K) * LN_EPS, None, op0=OP.add)
        std = small.tile([G, 1], f32)
        nc.scalar.sqrt(std, veps)
        rstd = small.tile([G, 1], f32)
        nc.vector.reciprocal(rstd, std)
        xn = cpool.tile([G, H], f32)
        nc.vector.tensor_scalar(
            xn, xsum, mv[:, 0:1], rstd[:, :1], op0=OP.subtract, op1=OP.mult
        )
        if trivial_affine:
            nc.sync.dma_start(out[:, :], xn)
        else:
            outt = cpool.tile([G, H], f32)
            nc.vector.tensor_mul(outt, xn, gb)
            nc.vector.tensor_add(outt, outt, bb)
            nc.sync.dma_start(out[:, :], outt)

    nc.compile()
    return nc


def core_inputs(core: int, x, attn, sp, xm, gamma, beta, trivial_affine=False) -> dict:
    """Per-core input map incl. host-precomputed constants."""
    sl = slice(core * G, (core + 1) * G)
    xmc = np.ascontiguousarray(xm[sl])  # [G, N]
    lens = xmc.sum(axis=1).astype(np.int32)  # [G]

    pidx = np.arange(P)
    # chunk masks: node p (c=0) / node 128+p (c=1) valid, interleaved pairs
    xmt2 = np.zeros((P, 2 * G), dtype=np.float32)
    xmt2[:, 0::2] = (pidx[:, None] < lens[None, :]).astype(np.float32)
    xmt2[:, 1::2] = ((pidx[:, None] + P) < lens[None, :]).astype(np.float32)
    # parity masks: node 2p / 2p+1 valid, in block-diagonal layout
    xme = (2 * pidx[:, None] < lens[None, :]).astype(np.float32)
    xmo = ((2 * pidx[:, None] + 1) < lens[None, :]).astype(np.float32)
    xbde = np.zeros((P, G * G), dtype=np.float32)
    xbdo = np.zeros((P, G * G), dtype=np.float32)
    for g in range(G):
        xbde[:, g * G + g] = xme[:, g]
        xbdo[:, g * G + g] = xmo[:, g]
    # NM: 0 on available nodes, -BIG on invalid + node 0
    nmpre = (xmc - 1.0) * BIG
    nmpre[:, 0] = -BIG
    iotaf = np.broadcast_to(
        np.arange(N, dtype=np.float32)[None, :], (G, N)
    ).copy()
    rowb = (np.arange(G, dtype=np.int64) * N).reshape(G, 1)
    ret_affine = {}
    if not trivial_affine:
        ret_affine = {
            "c_gb": np.broadcast_to(gamma.reshape(1, H), (G, H)).copy().astype(np.float32),
            "c_bb": np.broadcast_to(beta.reshape(1, H), (G, H)).copy().astype(np.float32),
        }
    return {
        **ret_affine,
        "x": np.ascontiguousarray(x[sl]),
        "attn": np.ascontiguousarray(attn[sl]),
        "spatial_pos": np.ascontiguousarray(sp[sl]),
        "x_mask": xmc,
        "c_xbde": xbde,
        "c_xbdo": xbdo,
        "c_xmt2": xmt2,
        "c_nmpre": nmpre.astype(np.float32),
        "c_iotaf": iotaf,
        "c_rowbi": rowb.astype(np.int32),
        "c_rowbf": rowb.astype(np.float32),
    }


_NC_CACHE = None


def kernel(**inputs) -> np.ndarray:
    global _NC_CACHE, LAST_RESULT
    from concourse.bass_utils import run_bass_kernel_spmd

    x = np.ascontiguousarray(np.asarray(inputs["x"]), dtype=np.float32)
    attn = np.ascontiguousarray(np.asarray(inputs["attn"]), dtype=np.float32)
    sp = np.ascontiguousarray(np.asarray(inputs["spatial_pos"]), dtype=np.float32)
    xm = np.ascontiguousarray(np.asarray(inputs["x_mask"]), dtype=np.float32)
    gamma = np.asarray(inputs["gamma"], dtype=np.float32)
    beta = np.asarray(inputs["beta"], dtype=np.float32)

    trivial = bool(np.all(gamma == 1.0) and np.all(beta == 0.0))
    if _NC_CACHE is None or _NC_CACHE[0] != trivial:
        _NC_CACHE = (trivial, build_bass(trivial_affine=trivial))
    nc = _NC_CACHE[1]

    in_maps = [
        core_inputs(c, x, attn, sp, xm, gamma, beta, trivial_affine=trivial)
        for c in range(NCORES)
    ]

    res = run_bass_kernel_spmd(
        nc, in_maps, core_ids=list(range(NCORES)), trace=TRACE
    )
    LAST_RESULT = res
    return np.concatenate([r["out"] for r in res.results], axis=0)
